# revision 49
# baseline (speedup 1.0000x reference)
"""Trainium2 Bass kernel for nn_BatchRankingLoss (pairwise ranking hinge loss).

Math: with o = squeeze(input), t = gdt_ts, B = 8192:
    loss = sum_{i,j} [|t_i - t_j| > 0.1] * relu(1 + sign(t_i - t_j)*(o_i - o_j)) / (B*(B-1))
By (i,j) <-> (j,i) symmetry this is exactly
    loss = 2 * sum_{(i,j): t_i - t_j > 0.1} relu(1 + o_i - o_j) / (B*(B-1)).

Rows are sorted by t on the host (a pure permutation; the pair sum is
permutation invariant), so the mask {j : t_i - t_j > 0.1} becomes a per-row
column prefix [0, K_i).  Rows are grouped into 64 tiles of 128 (contiguous in
sorted order) and dealt to the 8 cores round-robin per slot so every core gets
an identical instruction stream (SPMD) with near-identical work.

v2 design (per core, slot s covers columns [0, H_s), split at E_s):
  bulk [0, E_s): every row of the slot group is valid here.  DVE
      tensor_scalar(add bias, max 0) on bf16 -> h tiles; TensorE reduces
      (ones[128,1]^T @ h -> PSUM accumulate).  Some chunk pairs are folded
      (TT add) to shift work PE -> DVE for balance.
  band [E_s, H_s): data-dependent boundary.  The host ships ONE merged
      premasked fp8(e4m3) block covering all 8 slots' bands, with the
      per-(row,slot) bias DELTA baked into the data so a single bias vector
      (slot 0's) serves the whole block:
        band8[r, col(s,j)] = fp8(-o_j + bias[r,s] - bias[r,0]),  j < K_r
                           = -240 (relu(-240 + b) == 0)          otherwise
      The ACT engine consumes it in a few wide chunks:
      ACTIVATE(Relu, bias=bias[:,0], accum_out) at 1 elem/lane/cycle, fp8
      reads at full rate.
  nego (the shared -o row, bf16 [128, 6344]) is loaded with BROADCAST DMA:
      DRAM holds only the [1, 6344] row; the DMA descriptor replicates it to
      all 128 partitions (HBM reads drop 128x; matters with 8 cores sharing
      HBM).
Raw-Block implementation: hand-rolled semaphores, all input DMA issued as
early as possible (nego chunks on the Sync HWDGE queue, bias+band8 on the
Scalar queue before any ACT compute).
"""

import os
import sys

for _p in ("/opt/trn_rl_repo",):
    if _p not in sys.path:
        sys.path.insert(0, _p)

import numpy as np
import ml_dtypes

B = 8192
NCORES = 8
P = 128
NTILES = B // P            # 64
NSLOTS = NTILES // NCORES  # 8
GAP = np.float32(1.0)
THRESH = np.float32(0.1)
BIG_NEG8 = np.float32(-240.0)  # representable in e4m3; relu(-240+bias)==0

BF16 = ml_dtypes.bfloat16
FP8 = ml_dtypes.float8_e4m3

# tuning knobs
N_WARM_MM = int(os.environ.get("K_WARM_MM", "11"))
MM_N = 512
FOLD_PAIRS = int(os.environ.get("K_FOLD_PAIRS", "1"))   # folded chunk pairs
DVE_CHUNK = int(os.environ.get("K_DVE_CHUNK", "3584"))
FOLD_W = int(os.environ.get("K_FOLD_W", "1536"))        # width of fold halves
HRING = int(os.environ.get("K_HRING", "5"))
N_BAND_CHUNKS = int(os.environ.get("K_BAND_CHUNKS", "3"))
ACT_BULK = int(os.environ.get("K_ACT_BULK", "2048"))     # bulk cols for ACT
BCAST = os.environ.get("K_BCAST", "1") == "1"           # broadcast-DMA nego
# band slots consumed by the DVE+PE lane (premasked bf16) instead of ACT fp8
DVE_BAND_SLOTS = [int(x) for x in os.environ.get(
    "K_DVE_BANDS", "").split(",") if x != ""]
MM_N_B = int(os.environ.get("K_MM_N_B", "256"))  # tail PSUM bank width

# set after each run (when BASS_TRACE=1): HW exec time of the traced core
LAST_EXEC_NS = None


def _floor8(x):
    return (int(x) // 8) * 8


def _exact_prefix_counts(t_s):
    """K[i] = #{j : fp32(t_s[i] - t_s[j]) > 0.1}, exactly as fp32 computes it.

    t_s ascending => fp32(t_i - t_j) is non-increasing in j, so the counted set
    is the prefix [0, K[i]).
    """
    K = np.empty(B, dtype=np.int64)
    blk = 512
    for a in range(0, B, blk):
        b = min(a + blk, B)
        ld = (t_s[a:b, None] - t_s[None, :]).astype(np.float32)
        K[a:b] = (ld > THRESH).sum(axis=1)
    return K


def _geometry(K):
    K_lo = K[::P].reshape(NTILES)
    K_hi = K[P - 1::P].reshape(NTILES)
    E = np.empty(NSLOTS, dtype=np.int64)
    H = np.empty(NSLOTS, dtype=np.int64)
    for s in range(NSLOTS):
        tiles = [8 * s + c for c in range(NCORES)]
        E[s] = _floor8(min(K_lo[T] for T in tiles))
        H[s] = max(E[s], ((int(max(K_hi[T] for T in tiles)) + 7) // 8) * 8)
    return E, H


def _build_and_run(o_s, K):
    from contextlib import ExitStack

    import concourse.bacc as bacc
    import concourse.mybir as mybir
    from concourse.bass_utils import run_bass_kernel_spmd

    Alu = mybir.AluOpType
    F32 = mybir.dt.float32
    MBF16 = mybir.dt.bfloat16
    MFP8 = mybir.dt.float8e4
    RELU = mybir.ActivationFunctionType.Relu

    E, H = _geometry(K)
    W = H - E
    nego_cols = int(E.max())
    act_slots = [s for s in range(NSLOTS)
                 if W[s] > 0 and s not in DVE_BAND_SLOTS]
    dve_slots = [s for s in range(NSLOTS)
                 if W[s] > 0 and s in DVE_BAND_SLOTS]
    band_cols = int(sum(W[s] for s in act_slots))       # ACT fp8 block
    bandv_cols = int(sum(W[s] for s in dve_slots))      # DVE bf16 block
    band_off = {}
    off = 0
    for s in act_slots:
        band_off[s] = off
        off += int(W[s])
    bandv_off = {}
    off = 0
    for s in dve_slots:
        bandv_off[s] = off
        off += int(W[s])

    # nego DMA chunks: first one tiny (rides the Scalar queue in parallel
    # with bias, unblocking the DVE's first entry ~1us earlier); stream
    # SPLIT edges are coarser than DMA edges (wide TS ops amortize the
    # ~250ns DVE per-op overhead, gating handled by chunks_needed)
    edges = [int(x) for x in os.environ.get(
        "K_EDGES", "0,512,1280,2816,4608,99999").split(",")]
    edges = sorted({min(e, nego_cols) for e in edges})
    n_chunks = len(edges) - 1
    splits = [int(x) for x in os.environ.get(
        "K_SPLITS", "0,1024,2816,99999").split(",")]
    splits = sorted({min(e, nego_cols) for e in splits})
    n_splits = len(splits) - 1

    def chunks_needed(a, b):
        return [k for k in range(n_chunks) if edges[k] < b and edges[k + 1] > a]

    # ---- host-side inputs ----
    nego_bf = (-o_s).astype(BF16)
    if BCAST:
        nego_in = nego_bf[None, :nego_cols]
    else:
        nego_in = np.ascontiguousarray(
            np.broadcast_to(nego_bf[:nego_cols], (P, nego_cols)))

    in_maps = []
    for c in range(NCORES):
        bias = np.empty((P, NSLOTS), dtype=np.float32)
        for s in range(NSLOTS):
            rows0 = P * (8 * s + c)
            bias[:, s] = GAP + o_s[rows0:rows0 + P]
        # combo = [fp32 bias bit-pattern (2 bf16 cols per slot) |
        #          host-broadcast nego chunk0]: ONE descriptor so the DVE's
        # first entry clears a single DMA semaphore; the device reads the
        # bias back via a fp32 bitcast of the leading columns
        combo = np.empty((P, 2 * NSLOTS + edges[1]), dtype=BF16)
        combo[:, :2 * NSLOTS] = bias.view(np.uint16).view(BF16)
        combo[:, 2 * NSLOTS:] = np.broadcast_to(nego_bf[:edges[1]],
                                                (P, edges[1]))
        band8 = np.full((P, max(1, band_cols)), BIG_NEG8, dtype=np.float32)
        for s in act_slots:
            rows0 = P * (8 * s + c)
            idx = np.arange(E[s], H[s])
            valid = idx[None, :] < K[rows0:rows0 + P, None]
            # bias-delta baked in so one bias vector (slot 0's) serves all
            vals = (-o_s[idx][None, :]
                    + (bias[:, s] - bias[:, 0])[:, None]).astype(np.float32)
            band8[:, band_off[s]:band_off[s] + int(W[s])] = np.where(
                valid, vals, BIG_NEG8)
        bandv = np.full((P, max(1, bandv_cols)), -1000.0, dtype=BF16)
        for s in dve_slots:
            rows0 = P * (8 * s + c)
            idx = np.arange(E[s], H[s])
            valid = idx[None, :] < K[rows0:rows0 + P, None]
            bandv[:, bandv_off[s]:bandv_off[s] + int(W[s])] = np.where(
                valid, nego_bf[idx][None, :], BF16(-1000.0))
        im = {"nego": nego_in, "band8": band8.astype(FP8),
              "bandv": bandv, "combo": combo}
        in_maps.append(im)

    # ---- the DVE->PE tile stream (chunk-major: consume low columns of all
    # slots first so the stream never outruns the nego chunk arrivals) ----
    # entries: ("bulk", s, (a,b)) / ("fold", s, (a1,b1,a2,b2)) /
    #          ("bandv", s, (a,b))  [offsets into bandv block]
    stream = []
    folded_total = 0
    for k in range(n_splits):
        for s in range(NSLOTS):
            ca = max(splits[k], ACT_BULK if s == NSLOTS - 1 else 0)
            cb = min(splits[k + 1], int(E[s]))
            if cb <= ca:
                continue
            pos = ca
            if (k >= 1 and folded_total < FOLD_PAIRS
                    and cb - pos >= 2 * FOLD_W):
                stream.append(("fold", s, (pos, pos + FOLD_W,
                                           pos + FOLD_W, pos + 2 * FOLD_W)))
                folded_total += 1
                pos += 2 * FOLD_W
            while pos < cb:
                b = min(pos + DVE_CHUNK, cb)
                stream.append(("bulk", s, (pos, b)))
                pos = b
        if k == n_splits - 2:
            # bandv data lands mid-kernel; consume it before the last chunk
            # region so the stream (and the PE) finishes on bulk, letting the
            # final PSUM reduce overlap ACT's last band chunk
            for s in dve_slots:
                a = bandv_off[s]
                stream.append(("bandv", s, (a, a + int(W[s]))))
    n_tiles = len(stream)

    def entry_width(e):
        kind, s, span = e
        return span[1] - span[0]

    n_mmA = sum((entry_width(e) + MM_N - 1) // MM_N for e in stream[:-1])
    n_mmB = (entry_width(stream[-1]) + MM_N_B - 1) // MM_N_B

    # band chunk boundaries for ACT (align to slot edges where possible)
    act_edges_all = sorted({band_off[s] for s in act_slots} | {band_cols})
    bc_edges = [0]
    for i in range(1, N_BAND_CHUNKS):
        tgt = band_cols * i // N_BAND_CHUNKS
        snap = min((x for x in act_edges_all if x > 0),
                   key=lambda x: abs(int(x) - tgt))
        if int(snap) > bc_edges[-1]:
            bc_edges.append(int(snap))
    bc_edges.append(band_cols)
    bc_edges = sorted(set(bc_edges))
    n_bc = len(bc_edges) - 1
    act_bulk_spans = []
    pos = 0
    while pos < ACT_BULK:
        nxt = min(ACT_BULK, next((e for e in edges if e > pos), ACT_BULK))
        act_bulk_spans.append((pos, nxt))
        pos = nxt
    n_act = n_bc + len(act_bulk_spans)

    # ---- device program (raw Block, hand-rolled semaphores) ----
    nc = bacc.Bacc("TRN2", target_bir_lowering=False, debug=False)

    if BCAST:
        nego_d = nc.dram_tensor("nego", [1, nego_cols], MBF16,
                                kind="ExternalInput").ap()
    else:
        nego_d = nc.dram_tensor("nego", [P, nego_cols], MBF16,
                                kind="ExternalInput").ap()
    combo_d = nc.dram_tensor("combo", [P, 2 * NSLOTS + edges[1]], MBF16,
                             kind="ExternalInput").ap()
    band8_d = nc.dram_tensor("band8", [P, max(1, band_cols)], MFP8,
                             kind="ExternalInput").ap()
    bandv_d = nc.dram_tensor("bandv", [P, max(1, bandv_cols)], MBF16,
                             kind="ExternalInput").ap()
    NACC = 16
    acc_d = nc.dram_tensor("acc", [P, NACC], F32, kind="ExternalOutput").ap()

    with ExitStack() as ctx:
        ent_ = ctx.enter_context
        # leading 2*NSLOTS columns hold the fp32 bias bit-pattern (from
        # the combo DMA); nego column j lives at sbuf column 2*NSLOTS + j
        nego_sb = ent_(nc.sbuf_tensor("nego_sb", [P, 2 * NSLOTS + nego_cols],
                                      MBF16)).ap()
        band8_sb = ent_(nc.sbuf_tensor("band8_sb", [P, max(1, band_cols)],
                                       MFP8)).ap()
        bandv_sb = ent_(nc.sbuf_tensor("bandv_sb", [P, max(1, bandv_cols)],
                                       MBF16)).ap()
        acc_sb = ent_(nc.sbuf_tensor("acc_sb", [P, NACC], F32)).ap()
        warm_src = ent_(nc.sbuf_tensor("warm_src", [P, MM_N], MBF16)).ap()
        ones_sb = ent_(nc.sbuf_tensor("ones_sb", [P, 1], MBF16)).ap()
        act_scr = ent_(nc.sbuf_tensor(
            "act_scr", [P, max(ACT_BULK, 1, max(bc_edges[i + 1] - bc_edges[i]
                                                for i in range(n_bc)))],
            MBF16)).ap()
        hw_max = max(entry_width(e) for e in stream)
        h_ring = [ent_(nc.sbuf_tensor(f"h{r}", [P, hw_max], MBF16)).ap()
                  for r in range(HRING)]
        f_scr = [ent_(nc.sbuf_tensor(f"f{r}", [P, FOLD_W], MBF16)).ap()
                 for r in range(2)]

        warm_ps = ent_(nc.psum_tensor("warm_ps", [1, MM_N], F32)).ap()
        red_ps = ent_(nc.psum_tensor("red_ps", [1, MM_N], F32)).ap()
        red_psB = ent_(nc.psum_tensor("red_psB", [1, MM_N_B], F32)).ap()

        s_ng = [ent_(nc.semaphore(f"s_ng{k}")) for k in range(n_chunks)]
        s_bd = [ent_(nc.semaphore(f"s_bd{g}")) for g in range(n_bc)]
        s_bv = ent_(nc.semaphore("s_bv"))
        s_init = ent_(nc.semaphore("s_init"))
        s_h = ent_(nc.semaphore("s_h"))
        s_tile = ent_(nc.semaphore("s_tile"))
        s_actv = ent_(nc.semaphore("s_actv"))
        s_copy = ent_(nc.semaphore("s_copy"))
        s_out = ent_(nc.semaphore("s_out"))

        block = ent_(nc.Block(no_gpsimd_drain=True))

        class Tracker:
            def __init__(self, eng):
                self.eng = eng
                self.level = {}

            def need(self, sem, v):
                if v > self.level.get(id(sem), 0):
                    self.eng.wait_ge(sem, v)
                    self.level[id(sem)] = v

        @block.sync
        def _(sp):
            # all input DMA rides the Sync HWDGE queue (the Scalar queue
            # would eat ACT-lane time: each issue costs ~0.7us of engine).
            # Issue order == consumption order: nego chunk k feeds the DVE's
            # chunk-k region; band8 chunk g feeds ACT mid-kernel; bandv last.
            def ng(k):
                ca, cb = edges[k], edges[k + 1]
                if BCAST:
                    src = nego_d[:, ca:cb].broadcast_to([P, cb - ca])
                else:
                    src = nego_d[:, ca:cb]
                sp.dma_start(out=nego_sb[:, 2 * NSLOTS + ca:2 * NSLOTS + cb],
                             in_=src).then_inc(s_ng[k], 16)

            def bd(g):
                ba, bb = bc_edges[g], bc_edges[g + 1]
                sp.dma_start(out=band8_sb[:, ba:bb],
                             in_=band8_d[:, ba:bb]).then_inc(s_bd[g], 16)

            # chunk 0 (+bias) ride the Scalar queue in parallel; the Sync
            # queue starts at chunk 1 and interleaves band chunks
            order = []
            ngs = [("ng", k) for k in range(1, n_chunks)]
            bds = [("bd", g) for g in range(n_bc)]
            while ngs or bds:
                if ngs:
                    order.append(ngs.pop(0))
                if bds:
                    order.append(bds.pop(0))
            for kind, i in order:
                (ng if kind == "ng" else bd)(i)
            if bandv_cols > 0:
                sp.dma_start(out=bandv_sb[:], in_=bandv_d[:]) \
                    .then_inc(s_bv, 16)
            sp.wait_ge(s_actv, n_act)
            sp.wait_ge(s_copy, 2)
            sp.dma_start(out=acc_d[:], in_=acc_sb[:]).then_inc(s_out, 16)

        @block.scalar
        def _(sc):
            tr = Tracker(sc)
            # combo (bias-bf16 + nego chunk0) gates the DVE's first entry
            sc.dma_start(out=nego_sb[:, :2 * NSLOTS + edges[1]],
                         in_=combo_d[:]).then_inc(s_ng[0], 16)
            sc.wait_ge(s_init, 1)
            sc.activation(act_scr[:, :8], warm_src[:, :8], RELU, bias=0.0,
                          scale=1.0)
            # optional bulk lane: last slot's first ACT_BULK columns
            s7bias = nego_sb[:, 2 * (NSLOTS - 1):2 * NSLOTS].bitcast(F32)
            for i, (ba, bb) in enumerate(act_bulk_spans):
                for k in chunks_needed(ba, bb):
                    tr.need(s_ng[k], 16)
                sc.activation(act_scr[:, :bb - ba],
                              nego_sb[:, 2 * NSLOTS + ba:2 * NSLOTS + bb],
                              RELU, bias=s7bias, scale=1.0,
                              accum_out=acc_sb[:, n_bc + i:n_bc + i + 1]) \
                    .then_inc(s_actv, 1)
            # merged premasked fp8 band chunks, single shared bias (slot 0's)
            for g in range(n_bc):
                ba, bb = bc_edges[g], bc_edges[g + 1]
                tr.need(s_ng[0], 16)
                tr.need(s_bd[g], 16)
                sc.activation(act_scr[:, :bb - ba], band8_sb[:, ba:bb],
                              RELU, bias=nego_sb[:, 0:2].bitcast(F32),
                              scale=1.0,
                              accum_out=acc_sb[:, g:g + 1]) \
                    .then_inc(s_actv, 1)

        @block.vector
        def _(ve):
            tr = Tracker(ve)
            ve.memset(acc_sb[:], 0.0)
            ve.memset(warm_src[:], 0.0)
            # same-engine FIFO: this inc also implies warm_src is ready
            ve.memset(ones_sb[:], 1.0).then_inc(s_init, 1)
            for t, (kind, s, span) in enumerate(stream):
                if t >= HRING:
                    tr.need(s_tile, t - HRING + 1)
                h = h_ring[t % HRING]
                # fp32 bias scalars live (bit-cast) in the leading columns
                bias_ap = nego_sb[:, 2 * s:2 * s + 2].bitcast(F32)
                if kind == "bulk":
                    a, b = span
                    for k in chunks_needed(a, b):
                        tr.need(s_ng[k], 16)
                    ve.tensor_scalar(h[:, :b - a],
                                     nego_sb[:, 2 * NSLOTS + a:2 * NSLOTS + b],
                                     bias_ap,
                                     0.0, Alu.add, Alu.max).then_inc(s_h, 1)
                elif kind == "bandv":
                    a, b = span
                    tr.need(s_bv, 16)
                    ve.tensor_scalar(h[:, :b - a], bandv_sb[:, a:b], bias_ap,
                                     0.0, Alu.add, Alu.max).then_inc(s_h, 1)
                else:  # fold
                    a1, b1, a2, b2 = span
                    for k in chunks_needed(a1, b2):
                        tr.need(s_ng[k], 16)
                    ve.tensor_scalar(f_scr[0][:, :b1 - a1],
                                     nego_sb[:, 2 * NSLOTS + a1:2 * NSLOTS + b1],
                                     bias_ap, 0.0, Alu.add, Alu.max)
                    ve.tensor_scalar(f_scr[1][:, :b2 - a2],
                                     nego_sb[:, 2 * NSLOTS + a2:2 * NSLOTS + b2],
                                     bias_ap, 0.0, Alu.add, Alu.max)
                    ve.tensor_tensor(h[:, :b1 - a1], f_scr[0][:, :b1 - a1],
                                     f_scr[1][:, :b1 - a1], Alu.add) \
                        .then_inc(s_h, 1)
            # split reduce: A covers all but the last entry and overlaps the
            # PE's final matmuls; B (narrow bank) is the only serial tail
            ve.wait_ge(s_tile, n_tiles - 1)
            ve.tensor_reduce(acc_sb[0:1, NACC - 2:NACC - 1],
                             red_ps[:], mybir.AxisListType.X, Alu.add) \
                .then_inc(s_copy, 1)
            ve.wait_ge(s_tile, n_tiles)
            ve.tensor_reduce(acc_sb[0:1, NACC - 1:NACC],
                             red_psB[:], mybir.AxisListType.X, Alu.add) \
                .then_inc(s_copy, 1)

        @block.tensor
        def _(te):
            te.wait_ge(s_init, 1)
            for _ in range(N_WARM_MM):
                te.matmul(warm_ps[:], ones_sb[:], warm_src[:],
                          start=True, stop=True)
            mm_i = 0
            for t, e in enumerate(stream):
                width = entry_width(e)
                te.wait_ge(s_h, t + 1)
                h = h_ring[t % HRING]
                last_entry = (t == n_tiles - 1)
                mmn = MM_N_B if last_entry else MM_N
                ps = red_psB if last_entry else red_ps
                n_sub = (width + mmn - 1) // mmn
                for u in range(n_sub):
                    ma = u * mmn
                    mb = min(ma + mmn, width)
                    if last_entry:
                        start, stop = (u == 0), (u == n_sub - 1)
                    else:
                        start, stop = (mm_i == 0), (mm_i == n_mmA - 1)
                        mm_i += 1
                    mm = te.matmul(ps[:, :mb - ma], ones_sb[:],
                                   h[:, ma:mb], start=start, stop=stop,
                                   skip_group_check=True)
                    if u == n_sub - 1:
                        mm.then_inc(s_tile, 1)

    nc.compile()

    res = run_bass_kernel_spmd(nc, in_maps, core_ids=list(range(NCORES)))
    global LAST_EXEC_NS
    LAST_EXEC_NS = res.exec_time_ns
    if res.instructions_and_trace:
        print("trace:", res.instructions_and_trace[1])

    total_sum = 0.0
    for c in range(NCORES):
        r = res.results[c]
        acc = np.asarray(r["acc"]).astype(np.float64)
        total_sum += float(acc[0, NACC - 2])          # PE lane (PSUM A)
        total_sum += float(acc[0, NACC - 1])          # PE lane (PSUM B tail)
        total_sum += float(acc[:, :n_act].sum())      # ACT accumulators
    return total_sum


def kernel(input, gdt_ts):
    o = np.asarray(input, dtype=np.float32).reshape(B)
    t = np.asarray(gdt_ts, dtype=np.float32).reshape(B)

    perm = np.argsort(t, kind="stable")
    t_s = t[perm]
    o_s = o[perm]

    K = _exact_prefix_counts(t_s)

    total = _build_and_run(o_s, K)

    n_pairs = B * (B - 1)
    loss = np.float32(2.0 * total / n_pairs)
    return np.array([loss], dtype=np.float32)


if __name__ == "__main__":
    rng = np.random.default_rng(0)
    x = rng.standard_normal((B, 1)).astype(np.float32)
    ts = rng.random(B, dtype=np.float32)
    print(kernel(input=x, gdt_ts=ts))


# revision 51
# speedup vs baseline: 1.0283x; 1.0283x over previous
"""Trainium2 Bass kernel for nn_BatchRankingLoss (pairwise ranking hinge loss).

Math: with o = squeeze(input), t = gdt_ts, B = 8192:
    loss = sum_{i,j} [|t_i - t_j| > 0.1] * relu(1 + sign(t_i - t_j)*(o_i - o_j)) / (B*(B-1))
By (i,j) <-> (j,i) symmetry this is exactly
    loss = 2 * sum_{(i,j): t_i - t_j > 0.1} relu(1 + o_i - o_j) / (B*(B-1)).

Rows are sorted by t on the host (a pure permutation; the pair sum is
permutation invariant), so the mask {j : t_i - t_j > 0.1} becomes a per-row
column prefix [0, K_i).  Rows are grouped into 64 tiles of 128 (contiguous in
sorted order) and dealt to the 8 cores round-robin per slot so every core gets
an identical instruction stream (SPMD) with near-identical work.

Design (per core, slot s covers columns [0, H_s), split at E_s):
  bulk [0, E_s): every row of the slot group is valid here.  DVE
      tensor_scalar(add bias, max 0) on bf16 -> h tiles (chunk-major order
      so consumption never outruns DMA arrival); TensorE reduces each tile
      (ones[128,1]^T @ h) into an fp32 PSUM accumulation group.  The last
      stream entry goes to a second, narrow PSUM bank so the big reduce
      (PSUM A -> SBUF) overlaps the PE's final matmuls and only a ~0.4us
      reduce of bank B sits on the critical tail.
  band [E_s, H_s): data-dependent boundary.  The host ships ONE merged
      premasked fp8(e4m3) block covering all 8 slots' bands, with the
      per-(row,slot) bias DELTA baked into the data so a single bias vector
      (slot 0's) serves the whole block:
        band8[r, col(s,j)] = fp8(-o_j + bias[r,s] - bias[r,0]),  j < K_r
                           = -240 (relu(-240 + b) == 0)          otherwise
      The ACT engine consumes it in a few wide chunks (plus the first
      ~2048 bulk columns of the last slot, for 3-engine balance):
      ACTIVATE(Relu, bias, accum_out) at ~1 elem/lane/cycle; fp8 reads run
      at full rate.
  nego (the shared -o row, bf16 [128, 6344]) is loaded with BROADCAST DMA:
      DRAM holds only the [1, 6344] row; the DMA descriptor replicates it
      to all 128 partitions (HBM reads drop 128x; matters with 8 cores
      sharing HBM).
  bias rides inside the FIRST nego chunk ("combo" = fp32 bias bit-pattern
      stored as 16 leading bf16 columns + host-broadcast nego[0:512]): the
      DVE's and ACT's first ops clear a single DMA semaphore (~3us
      issue->completion pipeline per DMA, so one fewer gate on the
      critical path).  Engines read the bias via a fp32 bitcast AP.
Raw-Block implementation: hand-rolled semaphores; all input DMA issued
up-front (combo on the Scalar HWDGE queue in parallel with everything else
on the Sync queue, interleaved in consumption order); ~11 warmup matmuls
keep the PE clock ramped until real tiles arrive.
"""

import os
import sys

for _p in ("/opt/trn_rl_repo",):
    if _p not in sys.path:
        sys.path.insert(0, _p)

import numpy as np
import ml_dtypes

B = 8192
NCORES = 8
P = 128
NTILES = B // P            # 64
NSLOTS = NTILES // NCORES  # 8
GAP = np.float32(1.0)
THRESH = np.float32(0.1)
BIG_NEG8 = np.float32(-240.0)  # representable in e4m3; relu(-240+bias)==0

BF16 = ml_dtypes.bfloat16
FP8 = ml_dtypes.float8_e4m3

# tuning knobs
N_WARM_MM = int(os.environ.get("K_WARM_MM", "11"))
MM_N = 512
FOLD_PAIRS = int(os.environ.get("K_FOLD_PAIRS", "0"))   # folded chunk pairs
DVE_CHUNK = int(os.environ.get("K_DVE_CHUNK", "3584"))
FOLD_W = int(os.environ.get("K_FOLD_W", "1536"))        # width of fold halves
HRING = int(os.environ.get("K_HRING", "5"))
N_BAND_CHUNKS = int(os.environ.get("K_BAND_CHUNKS", "3"))
ACT_BULK = int(os.environ.get("K_ACT_BULK", "2048"))     # bulk cols for ACT
BCAST = os.environ.get("K_BCAST", "1") == "1"           # broadcast-DMA nego
# band slots consumed by the DVE+PE lane (premasked bf16) instead of ACT fp8
DVE_BAND_SLOTS = [int(x) for x in os.environ.get(
    "K_DVE_BANDS", "").split(",") if x != ""]
MM_N_B = int(os.environ.get("K_MM_N_B", "256"))  # tail PSUM bank width

# set after each run (when BASS_TRACE=1): HW exec time of the traced core
LAST_EXEC_NS = None


def _floor8(x):
    return (int(x) // 8) * 8


def _exact_prefix_counts(t_s):
    """K[i] = #{j : fp32(t_s[i] - t_s[j]) > 0.1}, exactly as fp32 computes it.

    t_s ascending => fp32(t_i - t_j) is non-increasing in j, so the counted set
    is the prefix [0, K[i]).
    """
    K = np.empty(B, dtype=np.int64)
    blk = 512
    for a in range(0, B, blk):
        b = min(a + blk, B)
        ld = (t_s[a:b, None] - t_s[None, :]).astype(np.float32)
        K[a:b] = (ld > THRESH).sum(axis=1)
    return K


def _geometry(K):
    K_lo = K[::P].reshape(NTILES)
    K_hi = K[P - 1::P].reshape(NTILES)
    E = np.empty(NSLOTS, dtype=np.int64)
    H = np.empty(NSLOTS, dtype=np.int64)
    for s in range(NSLOTS):
        tiles = [8 * s + c for c in range(NCORES)]
        E[s] = _floor8(min(K_lo[T] for T in tiles))
        H[s] = max(E[s], ((int(max(K_hi[T] for T in tiles)) + 7) // 8) * 8)
    return E, H


def _build_and_run(o_s, K):
    from contextlib import ExitStack

    import concourse.bacc as bacc
    import concourse.mybir as mybir
    from concourse.bass_utils import run_bass_kernel_spmd

    Alu = mybir.AluOpType
    F32 = mybir.dt.float32
    MBF16 = mybir.dt.bfloat16
    MFP8 = mybir.dt.float8e4
    RELU = mybir.ActivationFunctionType.Relu

    E, H = _geometry(K)
    W = H - E
    nego_cols = int(E.max())
    act_slots = [s for s in range(NSLOTS)
                 if W[s] > 0 and s not in DVE_BAND_SLOTS]
    dve_slots = [s for s in range(NSLOTS)
                 if W[s] > 0 and s in DVE_BAND_SLOTS]
    band_cols = int(sum(W[s] for s in act_slots))       # ACT fp8 block
    bandv_cols = int(sum(W[s] for s in dve_slots))      # DVE bf16 block
    band_off = {}
    off = 0
    for s in act_slots:
        band_off[s] = off
        off += int(W[s])
    bandv_off = {}
    off = 0
    for s in dve_slots:
        bandv_off[s] = off
        off += int(W[s])

    # nego DMA chunks: first one tiny (rides the Scalar queue in parallel
    # with bias, unblocking the DVE's first entry ~1us earlier); stream
    # SPLIT edges are coarser than DMA edges (wide TS ops amortize the
    # ~250ns DVE per-op overhead, gating handled by chunks_needed)
    edges = [int(x) for x in os.environ.get(
        "K_EDGES", "0,512,1280,2816,4608,99999").split(",")]
    edges = sorted({min(e, nego_cols) for e in edges})
    n_chunks = len(edges) - 1
    splits = [int(x) for x in os.environ.get(
        "K_SPLITS", "0,1024,2816,99999").split(",")]
    splits = sorted({min(e, nego_cols) for e in splits})
    n_splits = len(splits) - 1

    def chunks_needed(a, b):
        return [k for k in range(n_chunks) if edges[k] < b and edges[k + 1] > a]

    # ---- host-side inputs ----
    nego_bf = (-o_s).astype(BF16)
    if BCAST:
        nego_in = nego_bf[None, :nego_cols]
    else:
        nego_in = np.ascontiguousarray(
            np.broadcast_to(nego_bf[:nego_cols], (P, nego_cols)))

    in_maps = []
    for c in range(NCORES):
        bias = np.empty((P, NSLOTS), dtype=np.float32)
        for s in range(NSLOTS):
            rows0 = P * (8 * s + c)
            bias[:, s] = GAP + o_s[rows0:rows0 + P]
        # combo = [fp32 bias bit-pattern (2 bf16 cols per slot) |
        #          host-broadcast nego chunk0]: ONE descriptor so the DVE's
        # first entry clears a single DMA semaphore; the device reads the
        # bias back via a fp32 bitcast of the leading columns
        combo = np.empty((P, 2 * NSLOTS + edges[1]), dtype=BF16)
        combo[:, :2 * NSLOTS] = bias.view(np.uint16).view(BF16)
        combo[:, 2 * NSLOTS:] = np.broadcast_to(nego_bf[:edges[1]],
                                                (P, edges[1]))
        band8 = np.full((P, max(1, band_cols)), BIG_NEG8, dtype=np.float32)
        for s in act_slots:
            rows0 = P * (8 * s + c)
            idx = np.arange(E[s], H[s])
            valid = idx[None, :] < K[rows0:rows0 + P, None]
            # bias-delta baked in so one bias vector (slot 0's) serves all
            vals = (-o_s[idx][None, :]
                    + (bias[:, s] - bias[:, 0])[:, None]).astype(np.float32)
            band8[:, band_off[s]:band_off[s] + int(W[s])] = np.where(
                valid, vals, BIG_NEG8)
        bandv = np.full((P, max(1, bandv_cols)), -1000.0, dtype=BF16)
        for s in dve_slots:
            rows0 = P * (8 * s + c)
            idx = np.arange(E[s], H[s])
            valid = idx[None, :] < K[rows0:rows0 + P, None]
            bandv[:, bandv_off[s]:bandv_off[s] + int(W[s])] = np.where(
                valid, nego_bf[idx][None, :], BF16(-1000.0))
        im = {"nego": nego_in, "band8": band8.astype(FP8),
              "bandv": bandv, "combo": combo}
        in_maps.append(im)

    # ---- the DVE->PE tile stream (chunk-major: consume low columns of all
    # slots first so the stream never outruns the nego chunk arrivals) ----
    # entries: ("bulk", s, (a,b)) / ("fold", s, (a1,b1,a2,b2)) /
    #          ("bandv", s, (a,b))  [offsets into bandv block]
    stream = []
    folded_total = 0
    for k in range(n_splits):
        for s in range(NSLOTS):
            ca = max(splits[k], ACT_BULK if s == NSLOTS - 1 else 0)
            cb = min(splits[k + 1], int(E[s]))
            if cb <= ca:
                continue
            pos = ca
            if (k >= 1 and folded_total < FOLD_PAIRS
                    and cb - pos >= 2 * FOLD_W):
                stream.append(("fold", s, (pos, pos + FOLD_W,
                                           pos + FOLD_W, pos + 2 * FOLD_W)))
                folded_total += 1
                pos += 2 * FOLD_W
            while pos < cb:
                b = min(pos + DVE_CHUNK, cb)
                stream.append(("bulk", s, (pos, b)))
                pos = b
        if k == n_splits - 2:
            # bandv data lands mid-kernel; consume it before the last chunk
            # region so the stream (and the PE) finishes on bulk, letting the
            # final PSUM reduce overlap ACT's last band chunk
            for s in dve_slots:
                a = bandv_off[s]
                stream.append(("bandv", s, (a, a + int(W[s]))))
    n_tiles = len(stream)

    def entry_width(e):
        kind, s, span = e
        return span[1] - span[0]

    n_mmA = sum((entry_width(e) + MM_N - 1) // MM_N for e in stream[:-1])
    n_mmB = (entry_width(stream[-1]) + MM_N_B - 1) // MM_N_B

    # band chunk boundaries for ACT (align to slot edges where possible)
    act_edges_all = sorted({band_off[s] for s in act_slots} | {band_cols})
    bc_edges = [0]
    for i in range(1, N_BAND_CHUNKS):
        tgt = band_cols * i // N_BAND_CHUNKS
        snap = min((x for x in act_edges_all if x > 0),
                   key=lambda x: abs(int(x) - tgt))
        if int(snap) > bc_edges[-1]:
            bc_edges.append(int(snap))
    bc_edges.append(band_cols)
    bc_edges = sorted(set(bc_edges))
    n_bc = len(bc_edges) - 1
    act_bulk_spans = []
    pos = 0
    while pos < ACT_BULK:
        nxt = min(ACT_BULK, next((e for e in edges if e > pos), ACT_BULK))
        act_bulk_spans.append((pos, nxt))
        pos = nxt
    n_act = n_bc + len(act_bulk_spans)

    # ---- device program (raw Block, hand-rolled semaphores) ----
    nc = bacc.Bacc("TRN2", target_bir_lowering=False, debug=False)

    if BCAST:
        nego_d = nc.dram_tensor("nego", [1, nego_cols], MBF16,
                                kind="ExternalInput").ap()
    else:
        nego_d = nc.dram_tensor("nego", [P, nego_cols], MBF16,
                                kind="ExternalInput").ap()
    combo_d = nc.dram_tensor("combo", [P, 2 * NSLOTS + edges[1]], MBF16,
                             kind="ExternalInput").ap()
    band8_d = nc.dram_tensor("band8", [P, max(1, band_cols)], MFP8,
                             kind="ExternalInput").ap()
    bandv_d = nc.dram_tensor("bandv", [P, max(1, bandv_cols)], MBF16,
                             kind="ExternalInput").ap()
    NACC = 16
    acc_d = nc.dram_tensor("acc", [P, NACC], F32, kind="ExternalOutput").ap()

    with ExitStack() as ctx:
        ent_ = ctx.enter_context
        # leading 2*NSLOTS columns hold the fp32 bias bit-pattern (from
        # the combo DMA); nego column j lives at sbuf column 2*NSLOTS + j
        nego_sb = ent_(nc.sbuf_tensor("nego_sb", [P, 2 * NSLOTS + nego_cols],
                                      MBF16)).ap()
        band8_sb = ent_(nc.sbuf_tensor("band8_sb", [P, max(1, band_cols)],
                                       MFP8)).ap()
        bandv_sb = ent_(nc.sbuf_tensor("bandv_sb", [P, max(1, bandv_cols)],
                                       MBF16)).ap()
        acc_sb = ent_(nc.sbuf_tensor("acc_sb", [P, NACC], F32)).ap()
        warm_src = ent_(nc.sbuf_tensor("warm_src", [P, MM_N], MBF16)).ap()
        ones_sb = ent_(nc.sbuf_tensor("ones_sb", [P, 1], MBF16)).ap()
        act_scr = ent_(nc.sbuf_tensor(
            "act_scr", [P, max(ACT_BULK, 1, max(bc_edges[i + 1] - bc_edges[i]
                                                for i in range(n_bc)))],
            MBF16)).ap()
        hw_max = max(entry_width(e) for e in stream)
        h_ring = [ent_(nc.sbuf_tensor(f"h{r}", [P, hw_max], MBF16)).ap()
                  for r in range(HRING)]
        f_scr = [ent_(nc.sbuf_tensor(f"f{r}", [P, FOLD_W], MBF16)).ap()
                 for r in range(2)]

        warm_ps = ent_(nc.psum_tensor("warm_ps", [1, MM_N], F32)).ap()
        red_ps = ent_(nc.psum_tensor("red_ps", [1, MM_N], F32)).ap()
        red_psB = ent_(nc.psum_tensor("red_psB", [1, MM_N_B], F32)).ap()

        s_ng = [ent_(nc.semaphore(f"s_ng{k}")) for k in range(n_chunks)]
        s_bd = [ent_(nc.semaphore(f"s_bd{g}")) for g in range(n_bc)]
        s_bv = ent_(nc.semaphore("s_bv"))
        s_init = ent_(nc.semaphore("s_init"))
        s_h = ent_(nc.semaphore("s_h"))
        s_tile = ent_(nc.semaphore("s_tile"))
        s_actv = ent_(nc.semaphore("s_actv"))
        s_copy = ent_(nc.semaphore("s_copy"))
        s_out = ent_(nc.semaphore("s_out"))

        block = ent_(nc.Block(no_gpsimd_drain=True))

        class Tracker:
            def __init__(self, eng):
                self.eng = eng
                self.level = {}

            def need(self, sem, v):
                if v > self.level.get(id(sem), 0):
                    self.eng.wait_ge(sem, v)
                    self.level[id(sem)] = v

        @block.sync
        def _(sp):
            # all input DMA rides the Sync HWDGE queue (the Scalar queue
            # would eat ACT-lane time: each issue costs ~0.7us of engine).
            # Issue order == consumption order: nego chunk k feeds the DVE's
            # chunk-k region; band8 chunk g feeds ACT mid-kernel; bandv last.
            def ng(k):
                ca, cb = edges[k], edges[k + 1]
                if BCAST:
                    src = nego_d[:, ca:cb].broadcast_to([P, cb - ca])
                else:
                    src = nego_d[:, ca:cb]
                sp.dma_start(out=nego_sb[:, 2 * NSLOTS + ca:2 * NSLOTS + cb],
                             in_=src).then_inc(s_ng[k], 16)

            def bd(g):
                ba, bb = bc_edges[g], bc_edges[g + 1]
                sp.dma_start(out=band8_sb[:, ba:bb],
                             in_=band8_d[:, ba:bb]).then_inc(s_bd[g], 16)

            # chunk 0 (+bias) ride the Scalar queue in parallel; the Sync
            # queue starts at chunk 1 and interleaves band chunks
            order = []
            ngs = [("ng", k) for k in range(1, n_chunks)]
            bds = [("bd", g) for g in range(n_bc)]
            while ngs or bds:
                if ngs:
                    order.append(ngs.pop(0))
                if bds:
                    order.append(bds.pop(0))
            for kind, i in order:
                (ng if kind == "ng" else bd)(i)
            if bandv_cols > 0:
                sp.dma_start(out=bandv_sb[:], in_=bandv_d[:]) \
                    .then_inc(s_bv, 16)
            sp.wait_ge(s_actv, n_act)
            sp.wait_ge(s_copy, 2)
            sp.dma_start(out=acc_d[:], in_=acc_sb[:]).then_inc(s_out, 16)

        @block.scalar
        def _(sc):
            tr = Tracker(sc)
            # combo (bias-bf16 + nego chunk0) gates the DVE's first entry
            sc.dma_start(out=nego_sb[:, :2 * NSLOTS + edges[1]],
                         in_=combo_d[:]).then_inc(s_ng[0], 16)
            sc.wait_ge(s_init, 1)
            sc.activation(act_scr[:, :8], warm_src[:, :8], RELU, bias=0.0,
                          scale=1.0)
            # optional bulk lane: last slot's first ACT_BULK columns
            s7bias = nego_sb[:, 2 * (NSLOTS - 1):2 * NSLOTS].bitcast(F32)
            for i, (ba, bb) in enumerate(act_bulk_spans):
                for k in chunks_needed(ba, bb):
                    tr.need(s_ng[k], 16)
                sc.activation(act_scr[:, :bb - ba],
                              nego_sb[:, 2 * NSLOTS + ba:2 * NSLOTS + bb],
                              RELU, bias=s7bias, scale=1.0,
                              accum_out=acc_sb[:, n_bc + i:n_bc + i + 1]) \
                    .then_inc(s_actv, 1)
            # merged premasked fp8 band chunks, single shared bias (slot 0's)
            for g in range(n_bc):
                ba, bb = bc_edges[g], bc_edges[g + 1]
                tr.need(s_ng[0], 16)
                tr.need(s_bd[g], 16)
                sc.activation(act_scr[:, :bb - ba], band8_sb[:, ba:bb],
                              RELU, bias=nego_sb[:, 0:2].bitcast(F32),
                              scale=1.0,
                              accum_out=acc_sb[:, g:g + 1]) \
                    .then_inc(s_actv, 1)

        @block.vector
        def _(ve):
            tr = Tracker(ve)
            ve.memset(acc_sb[:], 0.0)
            ve.memset(warm_src[:], 0.0)
            # same-engine FIFO: this inc also implies warm_src is ready
            ve.memset(ones_sb[:], 1.0).then_inc(s_init, 1)
            for t, (kind, s, span) in enumerate(stream):
                if t >= HRING:
                    tr.need(s_tile, t - HRING + 1)
                h = h_ring[t % HRING]
                # fp32 bias scalars live (bit-cast) in the leading columns
                bias_ap = nego_sb[:, 2 * s:2 * s + 2].bitcast(F32)
                if kind == "bulk":
                    a, b = span
                    for k in chunks_needed(a, b):
                        tr.need(s_ng[k], 16)
                    ve.tensor_scalar(h[:, :b - a],
                                     nego_sb[:, 2 * NSLOTS + a:2 * NSLOTS + b],
                                     bias_ap,
                                     0.0, Alu.add, Alu.max).then_inc(s_h, 1)
                elif kind == "bandv":
                    a, b = span
                    tr.need(s_bv, 16)
                    ve.tensor_scalar(h[:, :b - a], bandv_sb[:, a:b], bias_ap,
                                     0.0, Alu.add, Alu.max).then_inc(s_h, 1)
                else:  # fold
                    a1, b1, a2, b2 = span
                    for k in chunks_needed(a1, b2):
                        tr.need(s_ng[k], 16)
                    ve.tensor_scalar(f_scr[0][:, :b1 - a1],
                                     nego_sb[:, 2 * NSLOTS + a1:2 * NSLOTS + b1],
                                     bias_ap, 0.0, Alu.add, Alu.max)
                    ve.tensor_scalar(f_scr[1][:, :b2 - a2],
                                     nego_sb[:, 2 * NSLOTS + a2:2 * NSLOTS + b2],
                                     bias_ap, 0.0, Alu.add, Alu.max)
                    ve.tensor_tensor(h[:, :b1 - a1], f_scr[0][:, :b1 - a1],
                                     f_scr[1][:, :b1 - a1], Alu.add) \
                        .then_inc(s_h, 1)
            # split reduce: A covers all but the last entry and overlaps the
            # PE's final matmuls; B (narrow bank) is the only serial tail
            ve.wait_ge(s_tile, n_tiles - 1)
            ve.tensor_reduce(acc_sb[0:1, NACC - 2:NACC - 1],
                             red_ps[:], mybir.AxisListType.X, Alu.add) \
                .then_inc(s_copy, 1)
            ve.wait_ge(s_tile, n_tiles)
            ve.tensor_reduce(acc_sb[0:1, NACC - 1:NACC],
                             red_psB[:], mybir.AxisListType.X, Alu.add) \
                .then_inc(s_copy, 1)

        @block.tensor
        def _(te):
            te.wait_ge(s_init, 1)
            for _ in range(N_WARM_MM):
                te.matmul(warm_ps[:], ones_sb[:], warm_src[:],
                          start=True, stop=True)
            mm_i = 0
            for t, e in enumerate(stream):
                width = entry_width(e)
                te.wait_ge(s_h, t + 1)
                h = h_ring[t % HRING]
                last_entry = (t == n_tiles - 1)
                mmn = MM_N_B if last_entry else MM_N
                ps = red_psB if last_entry else red_ps
                n_sub = (width + mmn - 1) // mmn
                for u in range(n_sub):
                    ma = u * mmn
                    mb = min(ma + mmn, width)
                    if last_entry:
                        start, stop = (u == 0), (u == n_sub - 1)
                    else:
                        start, stop = (mm_i == 0), (mm_i == n_mmA - 1)
                        mm_i += 1
                    mm = te.matmul(ps[:, :mb - ma], ones_sb[:],
                                   h[:, ma:mb], start=start, stop=stop,
                                   skip_group_check=True)
                    if u == n_sub - 1:
                        mm.then_inc(s_tile, 1)

    nc.compile()

    res = run_bass_kernel_spmd(nc, in_maps, core_ids=list(range(NCORES)))
    global LAST_EXEC_NS
    LAST_EXEC_NS = res.exec_time_ns
    if res.instructions_and_trace:
        print("trace:", res.instructions_and_trace[1])

    total_sum = 0.0
    for c in range(NCORES):
        r = res.results[c]
        acc = np.asarray(r["acc"]).astype(np.float64)
        total_sum += float(acc[0, NACC - 2])          # PE lane (PSUM A)
        total_sum += float(acc[0, NACC - 1])          # PE lane (PSUM B tail)
        total_sum += float(acc[:, :n_act].sum())      # ACT accumulators
    return total_sum


def kernel(input, gdt_ts):
    o = np.asarray(input, dtype=np.float32).reshape(B)
    t = np.asarray(gdt_ts, dtype=np.float32).reshape(B)

    perm = np.argsort(t, kind="stable")
    t_s = t[perm]
    o_s = o[perm]

    K = _exact_prefix_counts(t_s)

    total = _build_and_run(o_s, K)

    n_pairs = B * (B - 1)
    loss = np.float32(2.0 * total / n_pairs)
    return np.array([loss], dtype=np.float32)


if __name__ == "__main__":
    rng = np.random.default_rng(0)
    x = rng.standard_normal((B, 1)).astype(np.float32)
    ts = rng.random(B, dtype=np.float32)
    print(kernel(input=x, gdt_ts=ts))


# revision 53
# speedup vs baseline: 1.0307x; 1.0023x over previous
"""Trainium2 Bass kernel for nn_BatchRankingLoss (pairwise ranking hinge loss).

Math: with o = squeeze(input), t = gdt_ts, B = 8192:
    loss = sum_{i,j} [|t_i - t_j| > 0.1] * relu(1 + sign(t_i - t_j)*(o_i - o_j)) / (B*(B-1))
By (i,j) <-> (j,i) symmetry this is exactly
    loss = 2 * sum_{(i,j): t_i - t_j > 0.1} relu(1 + o_i - o_j) / (B*(B-1)).

Rows are sorted by t on the host (a pure permutation; the pair sum is
permutation invariant), so the mask {j : t_i - t_j > 0.1} becomes a per-row
column prefix [0, K_i).  Rows are grouped into 64 tiles of 128 (contiguous in
sorted order) and dealt to the 8 cores round-robin per slot so every core gets
an identical instruction stream (SPMD) with near-identical work.

Design (per core, slot s covers columns [0, H_s), split at E_s):
  bulk [0, E_s): every row of the slot group is valid here.  DVE
      tensor_scalar(add bias, max 0) on bf16 -> h tiles (chunk-major order
      so consumption never outruns DMA arrival); TensorE reduces each tile
      (ones[128,1]^T @ h) into an fp32 PSUM accumulation group.  The last
      stream entry goes to a second, narrow PSUM bank so the big reduce
      (PSUM A -> SBUF) overlaps the PE's final matmuls and only a ~0.4us
      reduce of bank B sits on the critical tail.
  band [E_s, H_s): data-dependent boundary.  The host ships ONE merged
      premasked fp8(e4m3) block covering all 8 slots' bands, with the
      per-(row,slot) bias DELTA baked into the data so a single bias vector
      (slot 0's) serves the whole block:
        band8[r, col(s,j)] = fp8(-o_j + bias[r,s] - bias[r,0]),  j < K_r
                           = -240 (relu(-240 + b) == 0)          otherwise
      The ACT engine consumes it in a few wide chunks (plus the first
      ~2048 bulk columns of the last slot, for 3-engine balance):
      ACTIVATE(Relu, bias, accum_out) at ~1 elem/lane/cycle; fp8 reads run
      at full rate.
  nego (the shared -o row, bf16 [128, 6344]) is loaded with BROADCAST DMA:
      DRAM holds only the [1, 6344] row; the DMA descriptor replicates it
      to all 128 partitions (HBM reads drop 128x; matters with 8 cores
      sharing HBM).
  bias rides inside the FIRST nego chunk ("combo" = fp32 bias bit-pattern
      stored as 16 leading bf16 columns + host-broadcast nego[0:512]): the
      DVE's and ACT's first ops clear a single DMA semaphore (~3us
      issue->completion pipeline per DMA, so one fewer gate on the
      critical path).  Engines read the bias via a fp32 bitcast AP.
Raw-Block implementation: hand-rolled semaphores; all input DMA issued
up-front (combo on the Scalar HWDGE queue in parallel with everything else
on the Sync queue, interleaved in consumption order); ~11 warmup matmuls
keep the PE clock ramped until real tiles arrive.
"""

import os
import sys

for _p in ("/opt/trn_rl_repo",):
    if _p not in sys.path:
        sys.path.insert(0, _p)

import numpy as np
import ml_dtypes

B = 8192
NCORES = 8
P = 128
NTILES = B // P            # 64
NSLOTS = NTILES // NCORES  # 8
GAP = np.float32(1.0)
THRESH = np.float32(0.1)
BIG_NEG8 = np.float32(-240.0)  # representable in e4m3; relu(-240+bias)==0

BF16 = ml_dtypes.bfloat16
FP8 = ml_dtypes.float8_e4m3

# tuning knobs
N_WARM_MM = int(os.environ.get("K_WARM_MM", "11"))
MM_N = 512
FOLD_PAIRS = int(os.environ.get("K_FOLD_PAIRS", "0"))   # folded chunk pairs
DVE_CHUNK = int(os.environ.get("K_DVE_CHUNK", "3584"))
FOLD_W = int(os.environ.get("K_FOLD_W", "1536"))        # width of fold halves
HRING = int(os.environ.get("K_HRING", "5"))
N_BAND_CHUNKS = int(os.environ.get("K_BAND_CHUNKS", "3"))
ACT_BULK = int(os.environ.get("K_ACT_BULK", "2048"))     # bulk cols for ACT
BCAST = os.environ.get("K_BCAST", "1") == "1"           # broadcast-DMA nego
# band slots consumed by the DVE+PE lane (premasked bf16) instead of ACT fp8
DVE_BAND_SLOTS = [int(x) for x in os.environ.get(
    "K_DVE_BANDS", "").split(",") if x != ""]
MM_N_B = int(os.environ.get("K_MM_N_B", "256"))  # tail PSUM bank width

# set after each run (when BASS_TRACE=1): HW exec time of the traced core
LAST_EXEC_NS = None


def _floor8(x):
    return (int(x) // 8) * 8


def _exact_prefix_counts(t_s):
    """K[i] = #{j : fp32(t_s[i] - t_s[j]) > 0.1}, exactly as fp32 computes it.

    t_s ascending => fp32(t_i - t_j) is non-increasing in j, so the counted set
    is the prefix [0, K[i]).
    """
    K = np.empty(B, dtype=np.int64)
    blk = 512
    for a in range(0, B, blk):
        b = min(a + blk, B)
        ld = (t_s[a:b, None] - t_s[None, :]).astype(np.float32)
        K[a:b] = (ld > THRESH).sum(axis=1)
    return K


def _geometry(K):
    K_lo = K[::P].reshape(NTILES)
    K_hi = K[P - 1::P].reshape(NTILES)
    E = np.empty(NSLOTS, dtype=np.int64)
    H = np.empty(NSLOTS, dtype=np.int64)
    for s in range(NSLOTS):
        tiles = [8 * s + c for c in range(NCORES)]
        E[s] = _floor8(min(K_lo[T] for T in tiles))
        H[s] = max(E[s], ((int(max(K_hi[T] for T in tiles)) + 7) // 8) * 8)
    return E, H


def _build_and_run(o_s, K):
    from contextlib import ExitStack

    import concourse.bacc as bacc
    import concourse.mybir as mybir
    from concourse.bass_utils import run_bass_kernel_spmd

    Alu = mybir.AluOpType
    F32 = mybir.dt.float32
    MBF16 = mybir.dt.bfloat16
    MFP8 = mybir.dt.float8e4
    RELU = mybir.ActivationFunctionType.Relu

    E, H = _geometry(K)
    W = H - E
    nego_cols = int(E.max())
    act_slots = [s for s in range(NSLOTS)
                 if W[s] > 0 and s not in DVE_BAND_SLOTS]
    dve_slots = [s for s in range(NSLOTS)
                 if W[s] > 0 and s in DVE_BAND_SLOTS]
    band_cols = int(sum(W[s] for s in act_slots))       # ACT fp8 block
    bandv_cols = int(sum(W[s] for s in dve_slots))      # DVE bf16 block
    band_off = {}
    off = 0
    for s in act_slots:
        band_off[s] = off
        off += int(W[s])
    bandv_off = {}
    off = 0
    for s in dve_slots:
        bandv_off[s] = off
        off += int(W[s])

    # nego DMA chunks: first one tiny (rides the Scalar queue in parallel
    # with bias, unblocking the DVE's first entry ~1us earlier); stream
    # SPLIT edges are coarser than DMA edges (wide TS ops amortize the
    # ~250ns DVE per-op overhead, gating handled by chunks_needed)
    edges = [int(x) for x in os.environ.get(
        "K_EDGES", "0,512,1280,2816,4608,99999").split(",")]
    edges = sorted({min(e, nego_cols) for e in edges})
    n_chunks = len(edges) - 1
    splits = [int(x) for x in os.environ.get(
        "K_SPLITS", "0,1024,2816,99999").split(",")]
    splits = sorted({min(e, nego_cols) for e in splits})
    n_splits = len(splits) - 1

    def chunks_needed(a, b):
        return [k for k in range(n_chunks) if edges[k] < b and edges[k + 1] > a]

    # ---- host-side inputs ----
    nego_bf = (-o_s).astype(BF16)
    if BCAST:
        nego_in = nego_bf[None, :nego_cols]
    else:
        nego_in = np.ascontiguousarray(
            np.broadcast_to(nego_bf[:nego_cols], (P, nego_cols)))

    in_maps = []
    for c in range(NCORES):
        bias = np.empty((P, NSLOTS), dtype=np.float32)
        for s in range(NSLOTS):
            rows0 = P * (8 * s + c)
            bias[:, s] = GAP + o_s[rows0:rows0 + P]
        # combo = [fp32 bias bit-pattern (2 bf16 cols per slot) |
        #          host-broadcast nego chunk0]: ONE descriptor so the DVE's
        # first entry clears a single DMA semaphore; the device reads the
        # bias back via a fp32 bitcast of the leading columns
        combo = np.empty((P, 2 * NSLOTS + edges[1]), dtype=BF16)
        combo[:, :2 * NSLOTS] = bias.view(np.uint16).view(BF16)
        combo[:, 2 * NSLOTS:] = np.broadcast_to(nego_bf[:edges[1]],
                                                (P, edges[1]))
        band8 = np.full((P, max(1, band_cols)), BIG_NEG8, dtype=np.float32)
        for s in act_slots:
            rows0 = P * (8 * s + c)
            idx = np.arange(E[s], H[s])
            valid = idx[None, :] < K[rows0:rows0 + P, None]
            # bias-delta baked in so one bias vector (slot 0's) serves all
            vals = (-o_s[idx][None, :]
                    + (bias[:, s] - bias[:, 0])[:, None]).astype(np.float32)
            band8[:, band_off[s]:band_off[s] + int(W[s])] = np.where(
                valid, vals, BIG_NEG8)
        bandv = np.full((P, max(1, bandv_cols)), -1000.0, dtype=BF16)
        for s in dve_slots:
            rows0 = P * (8 * s + c)
            idx = np.arange(E[s], H[s])
            valid = idx[None, :] < K[rows0:rows0 + P, None]
            bandv[:, bandv_off[s]:bandv_off[s] + int(W[s])] = np.where(
                valid, nego_bf[idx][None, :], BF16(-1000.0))
        im = {"nego": nego_in, "band8": band8.astype(FP8),
              "bandv": bandv, "combo": combo}
        in_maps.append(im)

    # ---- the DVE->PE tile stream (chunk-major: consume low columns of all
    # slots first so the stream never outruns the nego chunk arrivals) ----
    # entries: ("bulk", s, (a,b)) / ("fold", s, (a1,b1,a2,b2)) /
    #          ("bandv", s, (a,b))  [offsets into bandv block]
    # clamp the ACT bulk carve-out to the last slot's actual bulk width so
    # the kernel stays valid for any input distribution
    act_bulk = min(_floor8(ACT_BULK), int(E[NSLOTS - 1]))
    stream = []
    folded_total = 0
    for k in range(n_splits):
        for s in range(NSLOTS):
            ca = max(splits[k], act_bulk if s == NSLOTS - 1 else 0)
            cb = min(splits[k + 1], int(E[s]))
            if cb <= ca:
                continue
            pos = ca
            if (k >= 1 and folded_total < FOLD_PAIRS
                    and cb - pos >= 2 * FOLD_W):
                stream.append(("fold", s, (pos, pos + FOLD_W,
                                           pos + FOLD_W, pos + 2 * FOLD_W)))
                folded_total += 1
                pos += 2 * FOLD_W
            while pos < cb:
                b = min(pos + DVE_CHUNK, cb)
                stream.append(("bulk", s, (pos, b)))
                pos = b
        if k == n_splits - 2:
            # bandv data lands mid-kernel; consume it before the last chunk
            # region so the stream (and the PE) finishes on bulk, letting the
            # final PSUM reduce overlap ACT's last band chunk
            for s in dve_slots:
                a = bandv_off[s]
                stream.append(("bandv", s, (a, a + int(W[s]))))
    n_tiles = len(stream)

    def entry_width(e):
        kind, s, span = e
        return span[1] - span[0]

    n_mmA = sum((entry_width(e) + MM_N - 1) // MM_N for e in stream[:-1])
    n_mmB = (entry_width(stream[-1]) + MM_N_B - 1) // MM_N_B

    # band chunk boundaries for ACT (align to slot edges where possible)
    act_edges_all = sorted({band_off[s] for s in act_slots} | {band_cols})
    bc_edges = [0]
    for i in range(1, N_BAND_CHUNKS):
        tgt = band_cols * i // N_BAND_CHUNKS
        snap = min((x for x in act_edges_all if x > 0),
                   key=lambda x: abs(int(x) - tgt))
        if int(snap) > bc_edges[-1]:
            bc_edges.append(int(snap))
    bc_edges.append(band_cols)
    bc_edges = sorted(set(bc_edges))
    n_bc = len(bc_edges) - 1
    act_bulk_spans = []
    pos = 0
    while pos < act_bulk:
        nxt = min(act_bulk, next((e for e in edges if e > pos), act_bulk))
        act_bulk_spans.append((pos, nxt))
        pos = nxt
    n_act = n_bc + len(act_bulk_spans)

    # ---- device program (raw Block, hand-rolled semaphores) ----
    nc = bacc.Bacc("TRN2", target_bir_lowering=False, debug=False)

    if BCAST:
        nego_d = nc.dram_tensor("nego", [1, nego_cols], MBF16,
                                kind="ExternalInput").ap()
    else:
        nego_d = nc.dram_tensor("nego", [P, nego_cols], MBF16,
                                kind="ExternalInput").ap()
    combo_d = nc.dram_tensor("combo", [P, 2 * NSLOTS + edges[1]], MBF16,
                             kind="ExternalInput").ap()
    band8_d = nc.dram_tensor("band8", [P, max(1, band_cols)], MFP8,
                             kind="ExternalInput").ap()
    bandv_d = nc.dram_tensor("bandv", [P, max(1, bandv_cols)], MBF16,
                             kind="ExternalInput").ap()
    NACC = 16
    acc_d = nc.dram_tensor("acc", [P, NACC], F32, kind="ExternalOutput").ap()

    with ExitStack() as ctx:
        ent_ = ctx.enter_context
        # leading 2*NSLOTS columns hold the fp32 bias bit-pattern (from
        # the combo DMA); nego column j lives at sbuf column 2*NSLOTS + j
        nego_sb = ent_(nc.sbuf_tensor("nego_sb", [P, 2 * NSLOTS + nego_cols],
                                      MBF16)).ap()
        band8_sb = ent_(nc.sbuf_tensor("band8_sb", [P, max(1, band_cols)],
                                       MFP8)).ap()
        bandv_sb = ent_(nc.sbuf_tensor("bandv_sb", [P, max(1, bandv_cols)],
                                       MBF16)).ap()
        acc_sb = ent_(nc.sbuf_tensor("acc_sb", [P, NACC], F32)).ap()
        warm_src = ent_(nc.sbuf_tensor("warm_src", [P, MM_N], MBF16)).ap()
        ones_sb = ent_(nc.sbuf_tensor("ones_sb", [P, 1], MBF16)).ap()
        act_scr = ent_(nc.sbuf_tensor(
            "act_scr", [P, max(ACT_BULK, 1, max(bc_edges[i + 1] - bc_edges[i]
                                                for i in range(n_bc)))],
            MBF16)).ap()
        hw_max = max(entry_width(e) for e in stream)
        h_ring = [ent_(nc.sbuf_tensor(f"h{r}", [P, hw_max], MBF16)).ap()
                  for r in range(HRING)]
        f_scr = [ent_(nc.sbuf_tensor(f"f{r}", [P, FOLD_W], MBF16)).ap()
                 for r in range(2)]

        warm_ps = ent_(nc.psum_tensor("warm_ps", [1, MM_N], F32)).ap()
        red_ps = ent_(nc.psum_tensor("red_ps", [1, MM_N], F32)).ap()
        red_psB = ent_(nc.psum_tensor("red_psB", [1, MM_N_B], F32)).ap()

        s_ng = [ent_(nc.semaphore(f"s_ng{k}")) for k in range(n_chunks)]
        s_bd = [ent_(nc.semaphore(f"s_bd{g}")) for g in range(n_bc)]
        s_bv = ent_(nc.semaphore("s_bv"))
        s_init = ent_(nc.semaphore("s_init"))
        s_h = ent_(nc.semaphore("s_h"))
        s_tile = ent_(nc.semaphore("s_tile"))
        s_actv = ent_(nc.semaphore("s_actv"))
        s_copy = ent_(nc.semaphore("s_copy"))
        s_out = ent_(nc.semaphore("s_out"))

        block = ent_(nc.Block(no_gpsimd_drain=True))

        class Tracker:
            def __init__(self, eng):
                self.eng = eng
                self.level = {}

            def need(self, sem, v):
                if v > self.level.get(id(sem), 0):
                    self.eng.wait_ge(sem, v)
                    self.level[id(sem)] = v

        @block.sync
        def _(sp):
            # all input DMA rides the Sync HWDGE queue (the Scalar queue
            # would eat ACT-lane time: each issue costs ~0.7us of engine).
            # Issue order == consumption order: nego chunk k feeds the DVE's
            # chunk-k region; band8 chunk g feeds ACT mid-kernel; bandv last.
            def ng(k):
                ca, cb = edges[k], edges[k + 1]
                if BCAST:
                    src = nego_d[:, ca:cb].broadcast_to([P, cb - ca])
                else:
                    src = nego_d[:, ca:cb]
                sp.dma_start(out=nego_sb[:, 2 * NSLOTS + ca:2 * NSLOTS + cb],
                             in_=src).then_inc(s_ng[k], 16)

            def bd(g):
                ba, bb = bc_edges[g], bc_edges[g + 1]
                sp.dma_start(out=band8_sb[:, ba:bb],
                             in_=band8_d[:, ba:bb]).then_inc(s_bd[g], 16)

            # chunk 0 (+bias) ride the Scalar queue in parallel; the Sync
            # queue starts at chunk 1 and interleaves band chunks
            order = []
            ngs = [("ng", k) for k in range(1, n_chunks)]
            bds = [("bd", g) for g in range(n_bc)]
            while ngs or bds:
                if ngs:
                    order.append(ngs.pop(0))
                if bds:
                    order.append(bds.pop(0))
            for kind, i in order:
                (ng if kind == "ng" else bd)(i)
            if bandv_cols > 0:
                sp.dma_start(out=bandv_sb[:], in_=bandv_d[:]) \
                    .then_inc(s_bv, 16)
            sp.wait_ge(s_actv, n_act)
            sp.wait_ge(s_copy, 2)
            sp.dma_start(out=acc_d[:], in_=acc_sb[:]).then_inc(s_out, 16)

        @block.scalar
        def _(sc):
            tr = Tracker(sc)
            # combo (bias-bf16 + nego chunk0) gates the DVE's first entry
            sc.dma_start(out=nego_sb[:, :2 * NSLOTS + edges[1]],
                         in_=combo_d[:]).then_inc(s_ng[0], 16)
            sc.wait_ge(s_init, 1)
            sc.activation(act_scr[:, :8], warm_src[:, :8], RELU, bias=0.0,
                          scale=1.0)
            # optional bulk lane: last slot's first ACT_BULK columns
            s7bias = nego_sb[:, 2 * (NSLOTS - 1):2 * NSLOTS].bitcast(F32)
            for i, (ba, bb) in enumerate(act_bulk_spans):
                for k in chunks_needed(ba, bb):
                    tr.need(s_ng[k], 16)
                sc.activation(act_scr[:, :bb - ba],
                              nego_sb[:, 2 * NSLOTS + ba:2 * NSLOTS + bb],
                              RELU, bias=s7bias, scale=1.0,
                              accum_out=acc_sb[:, n_bc + i:n_bc + i + 1]) \
                    .then_inc(s_actv, 1)
            # merged premasked fp8 band chunks, single shared bias (slot 0's)
            for g in range(n_bc):
                ba, bb = bc_edges[g], bc_edges[g + 1]
                tr.need(s_ng[0], 16)
                tr.need(s_bd[g], 16)
                sc.activation(act_scr[:, :bb - ba], band8_sb[:, ba:bb],
                              RELU, bias=nego_sb[:, 0:2].bitcast(F32),
                              scale=1.0,
                              accum_out=acc_sb[:, g:g + 1]) \
                    .then_inc(s_actv, 1)

        @block.vector
        def _(ve):
            tr = Tracker(ve)
            ve.memset(acc_sb[:], 0.0)
            ve.memset(warm_src[:], 0.0)
            # same-engine FIFO: this inc also implies warm_src is ready
            ve.memset(ones_sb[:], 1.0).then_inc(s_init, 1)
            for t, (kind, s, span) in enumerate(stream):
                if t >= HRING:
                    tr.need(s_tile, t - HRING + 1)
                h = h_ring[t % HRING]
                # fp32 bias scalars live (bit-cast) in the leading columns
                bias_ap = nego_sb[:, 2 * s:2 * s + 2].bitcast(F32)
                if kind == "bulk":
                    a, b = span
                    for k in chunks_needed(a, b):
                        tr.need(s_ng[k], 16)
                    ve.tensor_scalar(h[:, :b - a],
                                     nego_sb[:, 2 * NSLOTS + a:2 * NSLOTS + b],
                                     bias_ap,
                                     0.0, Alu.add, Alu.max).then_inc(s_h, 1)
                elif kind == "bandv":
                    a, b = span
                    tr.need(s_bv, 16)
                    ve.tensor_scalar(h[:, :b - a], bandv_sb[:, a:b], bias_ap,
                                     0.0, Alu.add, Alu.max).then_inc(s_h, 1)
                else:  # fold
                    a1, b1, a2, b2 = span
                    for k in chunks_needed(a1, b2):
                        tr.need(s_ng[k], 16)
                    ve.tensor_scalar(f_scr[0][:, :b1 - a1],
                                     nego_sb[:, 2 * NSLOTS + a1:2 * NSLOTS + b1],
                                     bias_ap, 0.0, Alu.add, Alu.max)
                    ve.tensor_scalar(f_scr[1][:, :b2 - a2],
                                     nego_sb[:, 2 * NSLOTS + a2:2 * NSLOTS + b2],
                                     bias_ap, 0.0, Alu.add, Alu.max)
                    ve.tensor_tensor(h[:, :b1 - a1], f_scr[0][:, :b1 - a1],
                                     f_scr[1][:, :b1 - a1], Alu.add) \
                        .then_inc(s_h, 1)
            # split reduce: A covers all but the last entry and overlaps the
            # PE's final matmuls; B (narrow bank) is the only serial tail
            ve.wait_ge(s_tile, n_tiles - 1)
            ve.tensor_reduce(acc_sb[0:1, NACC - 2:NACC - 1],
                             red_ps[:], mybir.AxisListType.X, Alu.add) \
                .then_inc(s_copy, 1)
            ve.wait_ge(s_tile, n_tiles)
            ve.tensor_reduce(acc_sb[0:1, NACC - 1:NACC],
                             red_psB[:], mybir.AxisListType.X, Alu.add) \
                .then_inc(s_copy, 1)

        @block.tensor
        def _(te):
            te.wait_ge(s_init, 1)
            for _ in range(N_WARM_MM):
                te.matmul(warm_ps[:], ones_sb[:], warm_src[:],
                          start=True, stop=True)
            mm_i = 0
            for t, e in enumerate(stream):
                width = entry_width(e)
                te.wait_ge(s_h, t + 1)
                h = h_ring[t % HRING]
                last_entry = (t == n_tiles - 1)
                mmn = MM_N_B if last_entry else MM_N
                ps = red_psB if last_entry else red_ps
                n_sub = (width + mmn - 1) // mmn
                for u in range(n_sub):
                    ma = u * mmn
                    mb = min(ma + mmn, width)
                    if last_entry:
                        start, stop = (u == 0), (u == n_sub - 1)
                    else:
                        start, stop = (mm_i == 0), (mm_i == n_mmA - 1)
                        mm_i += 1
                    mm = te.matmul(ps[:, :mb - ma], ones_sb[:],
                                   h[:, ma:mb], start=start, stop=stop,
                                   skip_group_check=True)
                    if u == n_sub - 1:
                        mm.then_inc(s_tile, 1)

    nc.compile()

    res = run_bass_kernel_spmd(nc, in_maps, core_ids=list(range(NCORES)))
    global LAST_EXEC_NS
    LAST_EXEC_NS = res.exec_time_ns
    if res.instructions_and_trace:
        print("trace:", res.instructions_and_trace[1])

    total_sum = 0.0
    for c in range(NCORES):
        r = res.results[c]
        acc = np.asarray(r["acc"]).astype(np.float64)
        total_sum += float(acc[0, NACC - 2])          # PE lane (PSUM A)
        total_sum += float(acc[0, NACC - 1])          # PE lane (PSUM B tail)
        total_sum += float(acc[:, :n_act].sum())      # ACT accumulators
    return total_sum


def kernel(input, gdt_ts):
    o = np.asarray(input, dtype=np.float32).reshape(B)
    t = np.asarray(gdt_ts, dtype=np.float32).reshape(B)

    perm = np.argsort(t, kind="stable")
    t_s = t[perm]
    o_s = o[perm]

    K = _exact_prefix_counts(t_s)

    total = _build_and_run(o_s, K)

    n_pairs = B * (B - 1)
    loss = np.float32(2.0 * total / n_pairs)
    return np.array([loss], dtype=np.float32)


if __name__ == "__main__":
    rng = np.random.default_rng(0)
    x = rng.standard_normal((B, 1)).astype(np.float32)
    ts = rng.random(B, dtype=np.float32)
    print(kernel(input=x, gdt_ts=ts))
